# revision 25
# baseline (speedup 1.0000x reference)
"""DeepSeekMoE Trainium2 kernel (8 NeuronCores, SPMD).

Strategy (v2 — uniform item loop, no on-device gate):
  - Host computes top-2 routing and packs the routed tokens into exact-size
    per-expert groups (sum = T*K = 4096 columns, no capacity padding),
    then appends all T tokens once more as the "shared" group (2048 cols):
    one pairs matrix xp [D, 6144] = exactly 12 items of 512 columns.
  - The 8 shared experts (hidden FS=512 each, averaged) are algebraically
    one big FFN with hidden 8*512=4096; its per-core d_ff slice is exactly
    shared expert c.  So every core sees 9 uniform weight sets
    ([1024,512]/[1024,512]/[512,1024]): routed experts 0..7 (f-slice c)
    + shared expert c as weight-set 8.  alpha/NS (=1/16, exact) is folded
    into the shared w2 on the host.
  - Device: per item, ffn13 (w1/w3, 8 dt x 4 ft accumulation) -> silu*mul
    -> mm2 (w2) -> psum->sbuf bf16 copy -> DMA out.  Items may span expert
    boundaries; the per-(dt,ft) matmul is split at group boundaries
    (few ns extra per split).  mm2 is software-pipelined one item behind
    ffn13 so the silu/mul latency of the last ft never stalls the PE.
  - No gating on device: host multiplies routed outputs by (1-alpha)*prob
    during the combine (the standard MoE weighted-combine), so the PE does
    nothing but the 12*96 N=512 matmuls = the bf16 roofline.
  - Inputs stream on the Sync HWDGE queue in exact consumption order;
    outputs go on the Scalar HWDGE queue so their compute-gated waits can
    never head-of-line-block the input supply.
"""

import contextlib

import numpy as np
import ml_dtypes

import concourse.bacc as bacc
import concourse.tile as tile
import concourse.mybir as mybir
from concourse.bass_utils import run_bass_kernel_spmd

BF16 = ml_dtypes.bfloat16

B, S, D, F, E, NS, K = 2, 1024, 1024, 4096, 8, 8, 2
T = B * S
FS = F // NS            # shared expert hidden = 512 (= per-core routed slice)
FL = F // 8             # per-core f-slice of routed experts = 512
NW = E + 1              # 9 weight sets; wid 8 = shared
NCOL = T * K + T        # 6144 pair columns
IW = 512                # item width
NIT = NCOL // IW        # 12 items
ALPHA = 0.5
N_CORES = 8

# PE order of the 12 items: routed items r0..r7 occupy cols [512j, 512j+512),
# shared items s0..s3 are items 8..11 (cols 4096+).  Two shared items lead
# (tiny DMA footprint -> cheap prologue), the rest interleave for DMA slack.
PE_ORDER = [8, 9, 0, 1, 10, 2, 3, 11, 4, 5, 6, 7]

KCFG = {
    "warmup_mms": 44,    # garbage matmuls to warm the HAM clock gate and
                         # bridge the PE until the first input DMAs land
    "py_bufs": 3,
    "stream_last": 2,    # per-dt output DMA for the last N items
    "out_queue": "sync", # engine queue for output DMAs
}

_CACHE = {}         # sizes tuple -> compiled program
LAST_RESULT = None  # BassKernelResults of the most recent run (for profiling)


def _split_cost(sizes):
    """Extra PE ns caused by expert boundaries not landing on the 512 grid.

    Each of the 96 per-item (engine-step) rows issues one matmul per
    segment; a matmul costs max(25, N/2.4 + 2.5) ns, vs 215.8 for an
    unsplit 512 row.  Group sizes are all ~512+-30, so boundary offsets
    are small and the 60-cycle instruction floor dominates — the packing
    order of the groups controls how much of it we pay.
    """
    bounds = np.cumsum(sizes)
    tot = 0.0
    for j in range(8):
        lo, hi = 512 * j, 512 * j + 512
        cuts = [b for b in bounds[:-1] if lo < b < hi]
        pieces = np.diff([lo] + list(cuts) + [hi])
        tot += 96 * (sum(max(25.0, p / 2.4 + 2.5) for p in pieces)
                     - (512 / 2.4 + 2.5))
    return tot


def _best_perm(sizes):
    """Expert packing order minimizing the boundary split penalty
    (vectorized exhaustive search over all 8! orders)."""
    import itertools
    perms = np.array(list(itertools.permutations(range(E))), np.int64)
    s = np.asarray(sizes, np.int64)[perms]            # [P, 8]
    b = np.cumsum(s, axis=1)[:, :-1]                  # 7 boundaries
    d = (b % 512).astype(np.float64)
    pen = np.where(
        d > 0,
        np.maximum(25.0, d / 2.4 + 2.5)
        + np.maximum(25.0, (512 - d) / 2.4 + 2.5) - (512 / 2.4 + 2.5),
        0.0).sum(axis=1)
    return list(perms[int(np.argmin(pen))])


def _items_from_sizes(sizes):
    """Per-item segment lists [(wid, col_off_in_item, len), ...]."""
    bounds = np.cumsum([0] + list(sizes))
    items = []
    for j in range(8):                       # routed items
        lo, hi = IW * j, IW * j + IW
        segs = []
        for e in range(E):
            a, b = max(lo, int(bounds[e])), min(hi, int(bounds[e + 1]))
            if b > a:
                segs.append((e, a - lo, b - a))
        items.append(segs)
    for c in range(4):                       # shared items
        items.append([(E, 0, IW)])
    return items


def _build_program(sizes):
    bf = mybir.dt.bfloat16
    f32 = mybir.dt.float32
    Act = mybir.ActivationFunctionType

    items = _items_from_sizes(sizes)
    # first PE position at which each routed wid is used
    first_use = {}
    for pos, it in enumerate(PE_ORDER):
        for (wid, _, _) in items[it]:
            if wid != E and wid not in first_use:
                first_use[wid] = pos
    # weight-issue schedule: wid issued at block (first_use - 2)
    w_sched = {}
    for wid, fu in sorted(first_use.items(), key=lambda kv: kv[1]):
        w_sched.setdefault(max(0, fu - 2), []).append(wid)

    nc = bacc.Bacc("TRN2", target_bir_lowering=False, debug=False,
                   num_devices=N_CORES)

    # weights arrive host-packed in on-chip tile layout [128, a, free]:
    # each partition's data is contiguous in DRAM -> 8 KB DMA descriptors
    xp = nc.dram_tensor("xp", [D, NCOL], bf, kind="ExternalInput").ap()
    w1 = nc.dram_tensor("w1", [NW, 128, 8, FL], bf, kind="ExternalInput").ap()
    w3 = nc.dram_tensor("w3", [NW, 128, 8, FL], bf, kind="ExternalInput").ap()
    w2 = nc.dram_tensor("w2", [NW, 128, 4, D], bf, kind="ExternalInput").ap()
    y = nc.dram_tensor("y", [NIT, 128, 8, IW], bf, kind="ExternalOutput").ap()

    xp_r = xp.rearrange("(a p) t -> p a t", p=128)
    w1_r = [w1[i] for i in range(NW)]
    w3_r = [w3[i] for i in range(NW)]
    w2_r = [w2[i] for i in range(NW)]

    with tile.TileContext(nc) as tc:
        with contextlib.ExitStack() as ctx:
            const = ctx.enter_context(tc.tile_pool(name="const", bufs=1))
            wst = ctx.enter_context(tc.tile_pool(name="wst", bufs=4))
            acts = ctx.enter_context(tc.tile_pool(name="acts", bufs=4))
            hts = ctx.enter_context(tc.tile_pool(name="hts", bufs=2))
            spool = ctx.enter_context(tc.tile_pool(name="spool", bufs=2))
            outs = ctx.enter_context(tc.tile_pool(name="outs", bufs=2))
            psum = ctx.enter_context(
                tc.tile_pool(name="psum", bufs=2, space="PSUM"))
            psy = ctx.enter_context(
                tc.tile_pool(name="psy", bufs=KCFG["py_bufs"], space="PSUM"))
            psw = ctx.enter_context(
                tc.tile_pool(name="psw", bufs=1, space="PSUM"))

            state = {}
            out_dma = nc.sync.dma_start if KCFG["out_queue"] == "sync" \
                else nc.scalar.dma_start

            def load_w(wid):
                if wid == E:   # shared set: const pool, ft-split w1/w3
                    W1 = const.tile([128, 8, FL], bf, name="w1s")
                    W3 = const.tile([128, 8, FL], bf, name="w3s")
                    W2 = const.tile([128, 4, D], bf, name="w2s")
                else:
                    W1 = wst.tile([128, 8, FL], bf, tag="w1", name=f"w1_{wid}")
                    W3 = wst.tile([128, 8, FL], bf, tag="w3", name=f"w3_{wid}")
                    W2 = wst.tile([128, 4, D], bf, tag="w2", name=f"w2_{wid}")
                    nc.sync.dma_start(out=W1, in_=w1_r[wid])
                    nc.sync.dma_start(out=W3, in_=w3_r[wid])
                    nc.sync.dma_start(out=W2, in_=w2_r[wid])
                state[("W", wid)] = (W1, W3, W2)
                return W1, W3, W2

            def load_xp(pos):
                it = PE_ORDER[pos]
                o = it * IW
                XP = acts.tile([128, 8, IW], bf, tag="xp", name=f"xp{it}")
                nc.sync.dma_start(out=XP, in_=xp_r[:, :, o:o + IW])
                state[("XP", it)] = XP

            def ffn13(it):
                XP = state[("XP", it)]
                HT = hts.tile([128, 4, IW], bf, tag="ht", name=f"ht{it}")
                for ft in range(4):
                    fsl = slice(ft * 128, (ft + 1) * 128)
                    p1 = psum.tile([128, IW], f32, tag="p1", name=f"p1_{it}_{ft}")
                    p3 = psum.tile([128, IW], f32, tag="p3", name=f"p3_{it}_{ft}")
                    for wi, ps in ((0, p1), (1, p3)):
                        for (wid, o, ln) in items[it]:
                            W = state[("W", wid)][wi]
                            for dt in range(8):
                                nc.tensor.matmul(
                                    ps[:, o:o + ln], W[:, dt, fsl],
                                    XP[:, dt, o:o + ln],
                                    start=(dt == 0), stop=(dt == 7))
                    sa = spool.tile([128, IW], f32, tag="sa", name=f"sa{it}_{ft}")
                    nc.scalar.activation(sa, p1, Act.Silu)
                    nc.vector.tensor_mul(HT[:, ft, :], sa, p3)
                state[("HT", it)] = HT

            def mm2(it, stream_out=False):
                HT = state[("HT", it)]
                yo = outs.tile([128, 8, IW], bf, tag="yo", name=f"yo{it}")
                for dt in range(8):
                    dsl = slice(dt * 128, (dt + 1) * 128)
                    py = psy.tile([128, IW], f32, tag="py", name=f"py{it}_{dt}")
                    for (wid, o, ln) in items[it]:
                        W2t = state[("W", wid)][2]
                        for ft in range(4):
                            nc.tensor.matmul(
                                py[:, o:o + ln], W2t[:, ft, dsl],
                                HT[:, ft, o:o + ln],
                                start=(ft == 0), stop=(ft == 3))
                    if stream_out and dt == 7 and it == PE_ORDER[-1]:
                        # final block: halve the trailing cast+DMA chain
                        for h in range(2):
                            hs = slice(h * 256, h * 256 + 256)
                            nc.vector.tensor_copy(out=yo[:, dt, hs],
                                                  in_=py[:, hs])
                            out_dma(out=y[it, :, dt, hs], in_=yo[:, dt, hs])
                        continue
                    nc.vector.tensor_copy(out=yo[:, dt, :], in_=py)
                    if stream_out:
                        out_dma(out=y[it, :, dt, :], in_=yo[:, dt, :])
                if not stream_out:
                    out_dma(out=y[it], in_=yo)

            # ---- HAM warmup: garbage matmuls fill the DMA-dead window --
            # The PE clock gate (HAM) needs ~3.4us of sustained activity to
            # un-throttle from 1.2 to 2.4 GHz.  Data DMAs cannot land before
            # ~9us, so burn that window on matmuls over uninitialized SBUF;
            # by the time real matmuls issue, the PE is already warm.
            nwu = KCFG["warmup_mms"]
            if nwu:
                wub = const.tile([128, 128], bf, name="wub")
                wup = psw.tile([128, 128], f32, tag="wu", name="wup")
                nc.vector.memset(wub, 0.0)
                for i in range(nwu):
                    nc.tensor.matmul(wup, wub, wub, start=True, stop=True)

            # ---- prologue: DMAs in exact consumption order -------------
            it0 = PE_ORDER[0]
            W1S, W3S, W2S = load_w(E)
            XP0 = acts.tile([128, 8, IW], bf, tag="xp", name=f"xp{it0}")
            state[("XP", it0)] = XP0
            o0 = it0 * IW
            nc.sync.dma_start(out=W1S[:, :, 0:128], in_=w1_r[E][:, :, 0:128])
            for q in range(4):
                nc.sync.dma_start(out=XP0[:, 2 * q:2 * q + 2, :],
                                  in_=xp_r[:, 2 * q:2 * q + 2, o0:o0 + IW])
                if q == 1:
                    nc.sync.dma_start(out=W3S[:, :, 0:128],
                                      in_=w3_r[E][:, :, 0:128])
            for ft in range(1, 4):
                fsl = slice(ft * 128, (ft + 1) * 128)
                nc.sync.dma_start(out=W1S[:, :, fsl], in_=w1_r[E][:, :, fsl])
                nc.sync.dma_start(out=W3S[:, :, fsl], in_=w3_r[E][:, :, fsl])
            nc.sync.dma_start(out=W2S, in_=w2_r[E])
            load_xp(1)
            for wid in w_sched.get(0, []):
                load_w(wid)
            load_xp(2)

            # ---- main loop: mm2 lags ffn13 by one item -----------------
            nstream = KCFG["stream_last"]
            for pos in range(NIT):
                if pos >= 1:
                    for wid in w_sched.get(pos, []):
                        load_w(wid)
                    if pos + 2 < NIT:
                        load_xp(pos + 2)
                ffn13(PE_ORDER[pos])
                if pos >= 1:
                    mm2(PE_ORDER[pos - 1],
                        stream_out=(pos - 1 >= NIT - nstream))
            mm2(PE_ORDER[NIT - 1], stream_out=True)

    nc.compile()
    return nc


def kernel(hidden_states, gate_W, w1_e, w3_e, w2_e, w1_s, w3_s, w2_s):
    global LAST_RESULT
    x = np.ascontiguousarray(np.asarray(hidden_states, np.float32).reshape(T, D))

    # ---- host routing (sharding decision) + combine coefficients ----
    gate_W = np.asarray(gate_W, np.float32)
    logits = x @ gate_W.T                       # [T, E]
    m = logits.max(axis=1, keepdims=True)
    p = np.exp(logits - m)
    probs = p / p.sum(axis=1, keepdims=True)
    order = np.argsort(-probs, axis=1, kind="stable")[:, :K]   # [T, K]

    idx = [np.where((order == e).any(axis=1))[0] for e in range(E)]
    nsz = [len(te) for te in idx]
    assert sum(nsz) == T * K
    perm = _best_perm(nsz)                      # packing order of the groups
    sizes = tuple(nsz[e] for e in perm)

    # ---- build device inputs ----------------------------------------
    xT = np.ascontiguousarray(x.T)              # [D, T] fp32
    xf_bf = xT.astype(BF16)                     # [D, T]
    xp_bf = np.empty((D, NCOL), dtype=BF16)
    off = 0
    for e in perm:
        n = len(idx[e])
        xp_bf[:, off:off + n] = xf_bf[:, idx[e]]
        off += n
    xp_bf[:, T * K:] = xf_bf                    # shared group: all tokens

    w1_e = np.asarray(w1_e, np.float32)
    w3_e = np.asarray(w3_e, np.float32)
    w2_e = np.asarray(w2_e, np.float32)
    w1_s = np.asarray(w1_s, np.float32)
    w3_s = np.asarray(w3_s, np.float32)
    # fold alpha/NS (an exact power of two) into the shared down-proj
    w2_s = np.asarray(w2_s, np.float32) * (ALPHA / NS)

    nc = _CACHE.get(sizes)
    if nc is None:
        nc = _CACHE[sizes] = _build_program(sizes)

    def _pack(w, na):
        # [NW, na*128, free] -> tile layout [NW, 128, na, free], contiguous
        nw, dd, fr = w.shape
        return np.ascontiguousarray(
            w.reshape(nw, na, 128, fr).transpose(0, 2, 1, 3)).astype(BF16)

    in_maps = []
    for c in range(N_CORES):
        fsl = slice(c * FL, (c + 1) * FL)
        w1c = np.concatenate(
            [np.ascontiguousarray(w1_e[perm][:, :, fsl]), w1_s[c:c + 1]],
            axis=0)
        w3c = np.concatenate(
            [np.ascontiguousarray(w3_e[perm][:, :, fsl]), w3_s[c:c + 1]],
            axis=0)
        w2c = np.concatenate(
            [np.ascontiguousarray(w2_e[perm][:, fsl, :]), w2_s[c:c + 1]],
            axis=0)
        in_maps.append({
            "xp": xp_bf,
            "w1": _pack(w1c, 8),
            "w3": _pack(w3c, 8),
            "w2": _pack(w2c, 4),
        })

    res = run_bass_kernel_spmd(nc, in_maps, list(range(N_CORES)))
    LAST_RESULT = res

    # ---- host combine (unshard + weighted MoE combine) --------------
    yfull = np.zeros((NIT, 128, 8, IW), np.float32)
    for c in range(N_CORES):
        yfull += res.results[c]["y"].astype(np.float32)
    # [it, p, a, t] -> [a*128+p, it*512+t] = [D, NCOL]
    yfull = np.ascontiguousarray(yfull.transpose(2, 1, 0, 3)).reshape(D, NCOL)

    outT = yfull[:, T * K:].copy()              # shared part (scales folded)
    off = 0
    for e in perm:
        te = idx[e]
        coef = ((1.0 - ALPHA) * probs[te, e]).astype(np.float32)
        outT[:, te] += yfull[:, off:off + len(te)] * coef[None, :]
        off += len(te)

    return np.ascontiguousarray(outT.T).reshape(B, S, D).astype(np.float32)


# revision 28
# speedup vs baseline: 1.0053x; 1.0053x over previous
"""DeepSeekMoE Trainium2 kernel (8 NeuronCores, SPMD).

Strategy (v2 — uniform item loop, no on-device gate):
  - Host computes top-2 routing and packs the routed tokens into exact-size
    per-expert groups (sum = T*K = 4096 columns, no capacity padding),
    then appends all T tokens once more as the "shared" group (2048 cols):
    one pairs matrix xp [D, 6144] = exactly 12 items of 512 columns.
  - The 8 shared experts (hidden FS=512 each, averaged) are algebraically
    one big FFN with hidden 8*512=4096; its per-core d_ff slice is exactly
    shared expert c.  So every core sees 9 uniform weight sets
    ([1024,512]/[1024,512]/[512,1024]): routed experts 0..7 (f-slice c)
    + shared expert c as weight-set 8.  alpha/NS (=1/16, exact) is folded
    into the shared w2 on the host.
  - Device: per item, ffn13 (w1/w3, 8 dt x 4 ft accumulation) -> silu*mul
    -> mm2 (w2) -> psum->sbuf bf16 copy -> DMA out.  Items may span expert
    boundaries; the per-(dt,ft) matmul is split at group boundaries
    (few ns extra per split).  mm2 is software-pipelined one item behind
    ffn13 so the silu/mul latency of the last ft never stalls the PE.
  - No gating on device: host multiplies routed outputs by (1-alpha)*prob
    during the combine (the standard MoE weighted-combine), so the PE does
    nothing but the 12*96 N=512 matmuls = the bf16 roofline.
  - All DMAs ride the Sync HWDGE ring in exact consumption order (the
    scalar ring serializes against the Scalar engine's compute stream —
    measured regression).  A warmup burst of garbage matmuls fills the
    ~9us DMA-dead prologue and un-throttles the HAM clock gate before the
    first real matmul issues.
"""

import contextlib

import numpy as np
import ml_dtypes

import concourse.bacc as bacc
import concourse.tile as tile
import concourse.mybir as mybir
from concourse.bass_utils import run_bass_kernel_spmd

BF16 = ml_dtypes.bfloat16

B, S, D, F, E, NS, K = 2, 1024, 1024, 4096, 8, 8, 2
T = B * S
FS = F // NS            # shared expert hidden = 512 (= per-core routed slice)
FL = F // 8             # per-core f-slice of routed experts = 512
NW = E + 1              # 9 weight sets; wid 8 = shared
NCOL = T * K + T        # 6144 pair columns
IW = 512                # item width
NIT = NCOL // IW        # 12 items
ALPHA = 0.5
N_CORES = 8

# PE order of the 12 items: routed items r0..r7 occupy cols [512j, 512j+512),
# shared items s0..s3 are items 8..11 (cols 4096+).  Two shared items lead
# (tiny DMA footprint -> cheap prologue), the rest interleave for DMA slack.
PE_ORDER = [8, 9, 0, 1, 10, 2, 3, 11, 4, 5, 6, 7]

KCFG = {
    "warmup_mms": 56,    # garbage matmuls to warm the HAM clock gate and
                         # bridge the PE until the first input DMAs land
    "py_bufs": 3,
    "stream_last": 2,    # per-dt output DMA for the last N items
    "out_queue": "sync", # engine queue for output DMAs
}

_CACHE = {}         # sizes tuple -> compiled program
LAST_RESULT = None  # BassKernelResults of the most recent run (for profiling)


def _split_cost(sizes):
    """Extra PE ns caused by expert boundaries not landing on the 512 grid.

    Each of the 96 per-item (engine-step) rows issues one matmul per
    segment; a matmul costs max(25, N/2.4 + 2.5) ns, vs 215.8 for an
    unsplit 512 row.  Group sizes are all ~512+-30, so boundary offsets
    are small and the 60-cycle instruction floor dominates — the packing
    order of the groups controls how much of it we pay.
    """
    bounds = np.cumsum(sizes)
    tot = 0.0
    for j in range(8):
        lo, hi = 512 * j, 512 * j + 512
        cuts = [b for b in bounds[:-1] if lo < b < hi]
        pieces = np.diff([lo] + list(cuts) + [hi])
        tot += 96 * (sum(max(25.0, p / 2.4 + 2.5) for p in pieces)
                     - (512 / 2.4 + 2.5))
    return tot


def _best_perm(sizes):
    """Expert packing order minimizing the boundary split penalty
    (vectorized exhaustive search over all 8! orders)."""
    import itertools
    perms = np.array(list(itertools.permutations(range(E))), np.int64)
    s = np.asarray(sizes, np.int64)[perms]            # [P, 8]
    b = np.cumsum(s, axis=1)[:, :-1]                  # 7 boundaries
    d = (b % 512).astype(np.float64)
    pen = np.where(
        d > 0,
        np.maximum(25.0, d / 2.4 + 2.5)
        + np.maximum(25.0, (512 - d) / 2.4 + 2.5) - (512 / 2.4 + 2.5),
        0.0).sum(axis=1)
    return list(perms[int(np.argmin(pen))])


def _items_from_sizes(sizes):
    """Per-item segment lists [(wid, col_off_in_item, len), ...]."""
    bounds = np.cumsum([0] + list(sizes))
    items = []
    for j in range(8):                       # routed items
        lo, hi = IW * j, IW * j + IW
        segs = []
        for e in range(E):
            a, b = max(lo, int(bounds[e])), min(hi, int(bounds[e + 1]))
            if b > a:
                segs.append((e, a - lo, b - a))
        items.append(segs)
    for c in range(4):                       # shared items
        items.append([(E, 0, IW)])
    return items


def _build_program(sizes):
    bf = mybir.dt.bfloat16
    f32 = mybir.dt.float32
    Act = mybir.ActivationFunctionType

    items = _items_from_sizes(sizes)
    # first PE position at which each routed wid is used
    first_use = {}
    for pos, it in enumerate(PE_ORDER):
        for (wid, _, _) in items[it]:
            if wid != E and wid not in first_use:
                first_use[wid] = pos
    # weight-issue schedule: wid issued at block (first_use - 2)
    w_sched = {}
    for wid, fu in sorted(first_use.items(), key=lambda kv: kv[1]):
        w_sched.setdefault(max(0, fu - 2), []).append(wid)

    nc = bacc.Bacc("TRN2", target_bir_lowering=False, debug=False,
                   num_devices=N_CORES)

    # weights arrive host-packed in on-chip tile layout [128, a, free]:
    # each partition's data is contiguous in DRAM -> 8 KB DMA descriptors
    xp = nc.dram_tensor("xp", [D, NCOL], bf, kind="ExternalInput").ap()
    w1 = nc.dram_tensor("w1", [NW, 128, 8, FL], bf, kind="ExternalInput").ap()
    w3 = nc.dram_tensor("w3", [NW, 128, 8, FL], bf, kind="ExternalInput").ap()
    w2 = nc.dram_tensor("w2", [NW, 128, 4, D], bf, kind="ExternalInput").ap()
    y = nc.dram_tensor("y", [NIT, 128, 8, IW], bf, kind="ExternalOutput").ap()

    xp_r = xp.rearrange("(a p) t -> p a t", p=128)
    w1_r = [w1[i] for i in range(NW)]
    w3_r = [w3[i] for i in range(NW)]
    w2_r = [w2[i] for i in range(NW)]

    with tile.TileContext(nc) as tc:
        with contextlib.ExitStack() as ctx:
            const = ctx.enter_context(tc.tile_pool(name="const", bufs=1))
            wst = ctx.enter_context(tc.tile_pool(name="wst", bufs=4))
            acts = ctx.enter_context(tc.tile_pool(name="acts", bufs=4))
            hts = ctx.enter_context(tc.tile_pool(name="hts", bufs=2))
            spool = ctx.enter_context(tc.tile_pool(name="spool", bufs=2))
            outs = ctx.enter_context(tc.tile_pool(name="outs", bufs=2))
            psum = ctx.enter_context(
                tc.tile_pool(name="psum", bufs=2, space="PSUM"))
            psy = ctx.enter_context(
                tc.tile_pool(name="psy", bufs=KCFG["py_bufs"], space="PSUM"))
            psw = ctx.enter_context(
                tc.tile_pool(name="psw", bufs=1, space="PSUM"))

            state = {}
            out_dma = nc.sync.dma_start if KCFG["out_queue"] == "sync" \
                else nc.scalar.dma_start

            def load_w(wid):
                if wid == E:   # shared set: const pool, ft-split w1/w3
                    W1 = const.tile([128, 8, FL], bf, name="w1s")
                    W3 = const.tile([128, 8, FL], bf, name="w3s")
                    W2 = const.tile([128, 4, D], bf, name="w2s")
                else:
                    W1 = wst.tile([128, 8, FL], bf, tag="w1", name=f"w1_{wid}")
                    W3 = wst.tile([128, 8, FL], bf, tag="w3", name=f"w3_{wid}")
                    W2 = wst.tile([128, 4, D], bf, tag="w2", name=f"w2_{wid}")
                    nc.sync.dma_start(out=W1, in_=w1_r[wid])
                    nc.sync.dma_start(out=W3, in_=w3_r[wid])
                    nc.sync.dma_start(out=W2, in_=w2_r[wid])
                state[("W", wid)] = (W1, W3, W2)
                return W1, W3, W2

            def load_xp(pos):
                it = PE_ORDER[pos]
                o = it * IW
                XP = acts.tile([128, 8, IW], bf, tag="xp", name=f"xp{it}")
                nc.sync.dma_start(out=XP, in_=xp_r[:, :, o:o + IW])
                state[("XP", it)] = XP

            def ffn13(it):
                XP = state[("XP", it)]
                HT = hts.tile([128, 4, IW], bf, tag="ht", name=f"ht{it}")
                for ft in range(4):
                    fsl = slice(ft * 128, (ft + 1) * 128)
                    p1 = psum.tile([128, IW], f32, tag="p1", name=f"p1_{it}_{ft}")
                    p3 = psum.tile([128, IW], f32, tag="p3", name=f"p3_{it}_{ft}")
                    for wi, ps in ((0, p1), (1, p3)):
                        for (wid, o, ln) in items[it]:
                            W = state[("W", wid)][wi]
                            for dt in range(8):
                                nc.tensor.matmul(
                                    ps[:, o:o + ln], W[:, dt, fsl],
                                    XP[:, dt, o:o + ln],
                                    start=(dt == 0), stop=(dt == 7))
                    sa = spool.tile([128, IW], f32, tag="sa", name=f"sa{it}_{ft}")
                    nc.scalar.activation(sa, p1, Act.Silu)
                    nc.vector.tensor_mul(HT[:, ft, :], sa, p3)
                state[("HT", it)] = HT

            def mm2(it, stream_out=False):
                HT = state[("HT", it)]
                yo = outs.tile([128, 8, IW], bf, tag="yo", name=f"yo{it}")
                for dt in range(8):
                    dsl = slice(dt * 128, (dt + 1) * 128)
                    py = psy.tile([128, IW], f32, tag="py", name=f"py{it}_{dt}")
                    for (wid, o, ln) in items[it]:
                        W2t = state[("W", wid)][2]
                        for ft in range(4):
                            nc.tensor.matmul(
                                py[:, o:o + ln], W2t[:, ft, dsl],
                                HT[:, ft, o:o + ln],
                                start=(ft == 0), stop=(ft == 3))
                    nc.vector.tensor_copy(out=yo[:, dt, :], in_=py)
                    if stream_out:
                        out_dma(out=y[it, :, dt, :], in_=yo[:, dt, :])
                if not stream_out:
                    out_dma(out=y[it], in_=yo)

            # ---- HAM warmup: garbage matmuls fill the DMA-dead window --
            # The PE clock gate (HAM) needs ~3.4us of sustained activity to
            # un-throttle from 1.2 to 2.4 GHz.  Data DMAs cannot land before
            # ~9us, so burn that window on matmuls over uninitialized SBUF;
            # by the time real matmuls issue, the PE is already warm.
            nwu = KCFG["warmup_mms"]
            if nwu:
                wub = const.tile([128, 128], bf, name="wub")
                wup = psw.tile([128, 128], f32, tag="wu", name="wup")
                nc.vector.memset(wub, 0.0)
                for i in range(nwu):
                    nc.tensor.matmul(wup, wub, wub, start=True, stop=True)

            # ---- prologue: DMAs in exact consumption order -------------
            it0 = PE_ORDER[0]
            W1S, W3S, W2S = load_w(E)
            XP0 = acts.tile([128, 8, IW], bf, tag="xp", name=f"xp{it0}")
            state[("XP", it0)] = XP0
            o0 = it0 * IW
            nc.sync.dma_start(out=W1S[:, :, 0:128], in_=w1_r[E][:, :, 0:128])
            for q in range(4):
                nc.sync.dma_start(out=XP0[:, 2 * q:2 * q + 2, :],
                                  in_=xp_r[:, 2 * q:2 * q + 2, o0:o0 + IW])
                if q == 1:
                    nc.sync.dma_start(out=W3S[:, :, 0:128],
                                      in_=w3_r[E][:, :, 0:128])
            for ft in range(1, 4):
                fsl = slice(ft * 128, (ft + 1) * 128)
                nc.sync.dma_start(out=W1S[:, :, fsl], in_=w1_r[E][:, :, fsl])
                nc.sync.dma_start(out=W3S[:, :, fsl], in_=w3_r[E][:, :, fsl])
            nc.sync.dma_start(out=W2S, in_=w2_r[E])
            load_xp(1)
            for wid in w_sched.get(0, []):
                load_w(wid)
            load_xp(2)

            # ---- main loop: mm2 lags ffn13 by one item -----------------
            nstream = KCFG["stream_last"]
            for pos in range(NIT):
                if pos >= 1:
                    for wid in w_sched.get(pos, []):
                        load_w(wid)
                    if pos + 2 < NIT:
                        load_xp(pos + 2)
                ffn13(PE_ORDER[pos])
                if pos >= 1:
                    mm2(PE_ORDER[pos - 1],
                        stream_out=(pos - 1 >= NIT - nstream))
            mm2(PE_ORDER[NIT - 1], stream_out=True)

    nc.compile()
    return nc


def kernel(hidden_states, gate_W, w1_e, w3_e, w2_e, w1_s, w3_s, w2_s):
    global LAST_RESULT
    x = np.ascontiguousarray(np.asarray(hidden_states, np.float32).reshape(T, D))

    # ---- host routing (sharding decision) + combine coefficients ----
    gate_W = np.asarray(gate_W, np.float32)
    logits = x @ gate_W.T                       # [T, E]
    m = logits.max(axis=1, keepdims=True)
    p = np.exp(logits - m)
    probs = p / p.sum(axis=1, keepdims=True)
    order = np.argsort(-probs, axis=1, kind="stable")[:, :K]   # [T, K]

    idx = [np.where((order == e).any(axis=1))[0] for e in range(E)]
    nsz = [len(te) for te in idx]
    assert sum(nsz) == T * K
    perm = _best_perm(nsz)                      # packing order of the groups
    sizes = tuple(nsz[e] for e in perm)

    # ---- build device inputs ----------------------------------------
    xT = np.ascontiguousarray(x.T)              # [D, T] fp32
    xf_bf = xT.astype(BF16)                     # [D, T]
    xp_bf = np.empty((D, NCOL), dtype=BF16)
    off = 0
    for e in perm:
        n = len(idx[e])
        xp_bf[:, off:off + n] = xf_bf[:, idx[e]]
        off += n
    xp_bf[:, T * K:] = xf_bf                    # shared group: all tokens

    w1_e = np.asarray(w1_e, np.float32)
    w3_e = np.asarray(w3_e, np.float32)
    w2_e = np.asarray(w2_e, np.float32)
    w1_s = np.asarray(w1_s, np.float32)
    w3_s = np.asarray(w3_s, np.float32)
    # fold alpha/NS (an exact power of two) into the shared down-proj
    w2_s = np.asarray(w2_s, np.float32) * (ALPHA / NS)

    nc = _CACHE.get(sizes)
    if nc is None:
        nc = _CACHE[sizes] = _build_program(sizes)

    def _pack(w, na):
        # [NW, na*128, free] -> tile layout [NW, 128, na, free], contiguous
        nw, dd, fr = w.shape
        return np.ascontiguousarray(
            w.reshape(nw, na, 128, fr).transpose(0, 2, 1, 3)).astype(BF16)

    in_maps = []
    for c in range(N_CORES):
        fsl = slice(c * FL, (c + 1) * FL)
        w1c = np.concatenate(
            [np.ascontiguousarray(w1_e[perm][:, :, fsl]), w1_s[c:c + 1]],
            axis=0)
        w3c = np.concatenate(
            [np.ascontiguousarray(w3_e[perm][:, :, fsl]), w3_s[c:c + 1]],
            axis=0)
        w2c = np.concatenate(
            [np.ascontiguousarray(w2_e[perm][:, fsl, :]), w2_s[c:c + 1]],
            axis=0)
        in_maps.append({
            "xp": xp_bf,
            "w1": _pack(w1c, 8),
            "w3": _pack(w3c, 8),
            "w2": _pack(w2c, 4),
        })

    res = run_bass_kernel_spmd(nc, in_maps, list(range(N_CORES)))
    LAST_RESULT = res

    # ---- host combine (unshard + weighted MoE combine) --------------
    yfull = np.zeros((NIT, 128, 8, IW), np.float32)
    for c in range(N_CORES):
        yfull += res.results[c]["y"].astype(np.float32)
    # [it, p, a, t] -> [a*128+p, it*512+t] = [D, NCOL]
    yfull = np.ascontiguousarray(yfull.transpose(2, 1, 0, 3)).reshape(D, NCOL)

    outT = yfull[:, T * K:].copy()              # shared part (scales folded)
    off = 0
    for e in perm:
        te = idx[e]
        coef = ((1.0 - ALPHA) * probs[te, e]).astype(np.float32)
        outT[:, te] += yfull[:, off:off + len(te)] * coef[None, :]
        off += len(te)

    return np.ascontiguousarray(outT.T).reshape(B, S, D).astype(np.float32)


# revision 29
# speedup vs baseline: 1.0075x; 1.0023x over previous
"""DeepSeekMoE Trainium2 kernel (8 NeuronCores, SPMD).

Strategy (v2 — uniform item loop, no on-device gate):
  - Host computes top-2 routing and packs the routed tokens into exact-size
    per-expert groups (sum = T*K = 4096 columns, no capacity padding),
    then appends all T tokens once more as the "shared" group (2048 cols):
    one pairs matrix xp [D, 6144] = exactly 12 items of 512 columns.
  - The 8 shared experts (hidden FS=512 each, averaged) are algebraically
    one big FFN with hidden 8*512=4096; its per-core d_ff slice is exactly
    shared expert c.  So every core sees 9 uniform weight sets
    ([1024,512]/[1024,512]/[512,1024]): routed experts 0..7 (f-slice c)
    + shared expert c as weight-set 8.  alpha/NS (=1/16, exact) is folded
    into the shared w2 on the host.
  - Device: per item, ffn13 (w1/w3, 8 dt x 4 ft accumulation) -> silu*mul
    -> mm2 (w2) -> psum->sbuf bf16 copy -> DMA out.  Items may span expert
    boundaries; the per-(dt,ft) matmul is split at group boundaries
    (few ns extra per split).  mm2 is software-pipelined one item behind
    ffn13 so the silu/mul latency of the last ft never stalls the PE.
  - No gating on device: host multiplies routed outputs by (1-alpha)*prob
    during the combine (the standard MoE weighted-combine), so the PE does
    nothing but the 12*96 N=512 matmuls = the bf16 roofline.
  - All DMAs ride the Sync HWDGE ring in exact consumption order (the
    scalar ring serializes against the Scalar engine's compute stream —
    measured regression).  A warmup burst of garbage matmuls fills the
    ~9us DMA-dead prologue and un-throttles the HAM clock gate before the
    first real matmul issues.
"""

import contextlib

import numpy as np
import ml_dtypes

import concourse.bacc as bacc
import concourse.tile as tile
import concourse.mybir as mybir
from concourse.bass_utils import run_bass_kernel_spmd

BF16 = ml_dtypes.bfloat16

B, S, D, F, E, NS, K = 2, 1024, 1024, 4096, 8, 8, 2
T = B * S
FS = F // NS            # shared expert hidden = 512 (= per-core routed slice)
FL = F // 8             # per-core f-slice of routed experts = 512
NW = E + 1              # 9 weight sets; wid 8 = shared
NCOL = T * K + T        # 6144 pair columns
IW = 512                # item width
NIT = NCOL // IW        # 12 items
ALPHA = 0.5
N_CORES = 8

# PE order of the 12 items: routed items r0..r7 occupy cols [512j, 512j+512),
# shared items s0..s3 are items 8..11 (cols 4096+).  Two shared items lead
# (tiny DMA footprint -> cheap prologue), the rest interleave for DMA slack.
PE_ORDER = [8, 9, 0, 1, 10, 2, 3, 11, 4, 5, 6, 7]

KCFG = {
    "warmup_mms": 56,    # garbage matmuls to warm the HAM clock gate and
                         # bridge the PE until the first input DMAs land
    "py_bufs": 3,
    "stream_last": 2,    # per-dt output DMA for the last N items
    "out_queue": "sync", # engine queue for output DMAs
}

_CACHE = {}         # sizes tuple -> compiled program
LAST_RESULT = None  # BassKernelResults of the most recent run (for profiling)


def _split_cost(sizes):
    """Extra PE ns caused by expert boundaries not landing on the 512 grid.

    Each of the 96 per-item (engine-step) rows issues one matmul per
    segment; a matmul costs max(25, N/2.4 + 2.5) ns, vs 215.8 for an
    unsplit 512 row.  Group sizes are all ~512+-30, so boundary offsets
    are small and the 60-cycle instruction floor dominates — the packing
    order of the groups controls how much of it we pay.
    """
    bounds = np.cumsum(sizes)
    tot = 0.0
    for j in range(8):
        lo, hi = 512 * j, 512 * j + 512
        cuts = [b for b in bounds[:-1] if lo < b < hi]
        pieces = np.diff([lo] + list(cuts) + [hi])
        tot += 96 * (sum(max(25.0, p / 2.4 + 2.5) for p in pieces)
                     - (512 / 2.4 + 2.5))
    return tot


def _best_perm(sizes):
    """Expert packing order minimizing the boundary split penalty
    (vectorized exhaustive search over all 8! orders)."""
    import itertools
    perms = np.array(list(itertools.permutations(range(E))), np.int64)
    s = np.asarray(sizes, np.int64)[perms]            # [P, 8]
    b = np.cumsum(s, axis=1)[:, :-1]                  # 7 boundaries
    d = (b % 512).astype(np.float64)
    pen = np.where(
        d > 0,
        np.maximum(25.0, d / 2.4 + 2.5)
        + np.maximum(25.0, (512 - d) / 2.4 + 2.5) - (512 / 2.4 + 2.5),
        0.0).sum(axis=1)
    return list(perms[int(np.argmin(pen))])


def _items_from_sizes(sizes):
    """Per-item segment lists [(wid, col_off_in_item, len), ...]."""
    bounds = np.cumsum([0] + list(sizes))
    items = []
    for j in range(8):                       # routed items
        lo, hi = IW * j, IW * j + IW
        segs = []
        for e in range(E):
            a, b = max(lo, int(bounds[e])), min(hi, int(bounds[e + 1]))
            if b > a:
                segs.append((e, a - lo, b - a))
        items.append(segs)
    for c in range(4):                       # shared items
        items.append([(E, 0, IW)])
    return items


def _build_program(sizes):
    bf = mybir.dt.bfloat16
    f32 = mybir.dt.float32
    Act = mybir.ActivationFunctionType

    items = _items_from_sizes(sizes)
    # first PE position at which each routed wid is used
    first_use = {}
    for pos, it in enumerate(PE_ORDER):
        for (wid, _, _) in items[it]:
            if wid != E and wid not in first_use:
                first_use[wid] = pos
    # weight-issue schedule: wid issued at block (first_use - 2)
    w_sched = {}
    for wid, fu in sorted(first_use.items(), key=lambda kv: kv[1]):
        w_sched.setdefault(max(0, fu - 2), []).append(wid)

    nc = bacc.Bacc("TRN2", target_bir_lowering=False, debug=False,
                   num_devices=N_CORES)

    # weights arrive host-packed in on-chip tile layout [128, a, free]:
    # each partition's data is contiguous in DRAM -> 8 KB DMA descriptors
    xp = nc.dram_tensor("xp", [D, NCOL], bf, kind="ExternalInput").ap()
    w1 = nc.dram_tensor("w1", [NW, 128, 8, FL], bf, kind="ExternalInput").ap()
    w3 = nc.dram_tensor("w3", [NW, 128, 8, FL], bf, kind="ExternalInput").ap()
    w2 = nc.dram_tensor("w2", [NW, 128, 4, D], bf, kind="ExternalInput").ap()
    y = nc.dram_tensor("y", [NIT, 128, 8, IW], bf, kind="ExternalOutput").ap()

    xp_r = xp.rearrange("(a p) t -> p a t", p=128)
    w1_r = [w1[i] for i in range(NW)]
    w3_r = [w3[i] for i in range(NW)]
    w2_r = [w2[i] for i in range(NW)]

    with tile.TileContext(nc) as tc:
        with contextlib.ExitStack() as ctx:
            const = ctx.enter_context(tc.tile_pool(name="const", bufs=1))
            wst = ctx.enter_context(tc.tile_pool(name="wst", bufs=4))
            acts = ctx.enter_context(tc.tile_pool(name="acts", bufs=4))
            hts = ctx.enter_context(tc.tile_pool(name="hts", bufs=2))
            spool = ctx.enter_context(tc.tile_pool(name="spool", bufs=2))
            outs = ctx.enter_context(tc.tile_pool(name="outs", bufs=2))
            psum = ctx.enter_context(
                tc.tile_pool(name="psum", bufs=2, space="PSUM"))
            psy = ctx.enter_context(
                tc.tile_pool(name="psy", bufs=KCFG["py_bufs"], space="PSUM"))
            psw = ctx.enter_context(
                tc.tile_pool(name="psw", bufs=1, space="PSUM"))

            state = {}
            out_dma = nc.sync.dma_start if KCFG["out_queue"] == "sync" \
                else nc.scalar.dma_start

            def load_w(wid):
                if wid == E:   # shared set: const pool, ft-split w1/w3
                    W1 = const.tile([128, 8, FL], bf, name="w1s")
                    W3 = const.tile([128, 8, FL], bf, name="w3s")
                    W2 = const.tile([128, 4, D], bf, name="w2s")
                else:
                    W1 = wst.tile([128, 8, FL], bf, tag="w1", name=f"w1_{wid}")
                    W3 = wst.tile([128, 8, FL], bf, tag="w3", name=f"w3_{wid}")
                    W2 = wst.tile([128, 4, D], bf, tag="w2", name=f"w2_{wid}")
                    nc.sync.dma_start(out=W1, in_=w1_r[wid])
                    nc.sync.dma_start(out=W3, in_=w3_r[wid])
                    nc.sync.dma_start(out=W2, in_=w2_r[wid])
                state[("W", wid)] = (W1, W3, W2)
                return W1, W3, W2

            def load_xp(pos):
                it = PE_ORDER[pos]
                o = it * IW
                XP = acts.tile([128, 8, IW], bf, tag="xp", name=f"xp{it}")
                nc.sync.dma_start(out=XP, in_=xp_r[:, :, o:o + IW])
                state[("XP", it)] = XP

            def ffn13(it):
                XP = state[("XP", it)]
                HT = hts.tile([128, 4, IW], bf, tag="ht", name=f"ht{it}")
                for ft in range(4):
                    fsl = slice(ft * 128, (ft + 1) * 128)
                    p1 = psum.tile([128, IW], f32, tag="p1", name=f"p1_{it}_{ft}")
                    p3 = psum.tile([128, IW], f32, tag="p3", name=f"p3_{it}_{ft}")
                    for wi, ps in ((0, p1), (1, p3)):
                        for (wid, o, ln) in items[it]:
                            W = state[("W", wid)][wi]
                            for dt in range(8):
                                nc.tensor.matmul(
                                    ps[:, o:o + ln], W[:, dt, fsl],
                                    XP[:, dt, o:o + ln],
                                    start=(dt == 0), stop=(dt == 7))
                    sa = spool.tile([128, IW], f32, tag="sa", name=f"sa{it}_{ft}")
                    nc.scalar.activation(sa, p1, Act.Silu)
                    nc.vector.tensor_mul(HT[:, ft, :], sa, p3)
                state[("HT", it)] = HT

            def mm2(it, stream_out=False):
                HT = state[("HT", it)]
                yo = outs.tile([128, 8, IW], bf, tag="yo", name=f"yo{it}")
                for dt in range(8):
                    dsl = slice(dt * 128, (dt + 1) * 128)
                    py = psy.tile([128, IW], f32, tag="py", name=f"py{it}_{dt}")
                    for (wid, o, ln) in items[it]:
                        W2t = state[("W", wid)][2]
                        for ft in range(4):
                            nc.tensor.matmul(
                                py[:, o:o + ln], W2t[:, ft, dsl],
                                HT[:, ft, o:o + ln],
                                start=(ft == 0), stop=(ft == 3))
                    nc.vector.tensor_copy(out=yo[:, dt, :], in_=py)
                    if stream_out:
                        out_dma(out=y[it, :, dt, :], in_=yo[:, dt, :])
                if not stream_out:
                    out_dma(out=y[it], in_=yo)

            # ---- HAM warmup: garbage matmuls fill the DMA-dead window --
            # The PE clock gate (HAM) needs ~3.4us of sustained activity to
            # un-throttle from 1.2 to 2.4 GHz.  Data DMAs cannot land before
            # ~9us, so burn that window on matmuls over uninitialized SBUF;
            # by the time real matmuls issue, the PE is already warm.
            nwu = KCFG["warmup_mms"]
            if nwu:
                wub = const.tile([128, 128], bf, name="wub")
                wup = psw.tile([128, 128], f32, tag="wu", name="wup")
                nc.vector.memset(wub, 0.0)
                for i in range(nwu):
                    nc.tensor.matmul(wup, wub, wub, start=True, stop=True)

            # ---- prologue: DMAs in exact consumption order -------------
            it0 = PE_ORDER[0]
            W1S, W3S, W2S = load_w(E)
            XP0 = acts.tile([128, 8, IW], bf, tag="xp", name=f"xp{it0}")
            state[("XP", it0)] = XP0
            o0 = it0 * IW
            nc.sync.dma_start(out=W1S[:, :, 0:128], in_=w1_r[E][:, :, 0:128])
            for q in range(4):
                nc.sync.dma_start(out=XP0[:, 2 * q:2 * q + 2, :],
                                  in_=xp_r[:, 2 * q:2 * q + 2, o0:o0 + IW])
            nc.sync.dma_start(out=W3S[:, :, 0:128], in_=w3_r[E][:, :, 0:128])
            for ft in range(1, 4):
                fsl = slice(ft * 128, (ft + 1) * 128)
                nc.sync.dma_start(out=W1S[:, :, fsl], in_=w1_r[E][:, :, fsl])
                nc.sync.dma_start(out=W3S[:, :, fsl], in_=w3_r[E][:, :, fsl])
            nc.sync.dma_start(out=W2S, in_=w2_r[E])
            load_xp(1)
            for wid in w_sched.get(0, []):
                load_w(wid)
            load_xp(2)

            # ---- main loop: mm2 lags ffn13 by one item -----------------
            nstream = KCFG["stream_last"]
            for pos in range(NIT):
                if pos >= 1:
                    for wid in w_sched.get(pos, []):
                        load_w(wid)
                    if pos + 2 < NIT:
                        load_xp(pos + 2)
                ffn13(PE_ORDER[pos])
                if pos >= 1:
                    mm2(PE_ORDER[pos - 1],
                        stream_out=(pos - 1 >= NIT - nstream))
            mm2(PE_ORDER[NIT - 1], stream_out=True)

    nc.compile()
    return nc


def kernel(hidden_states, gate_W, w1_e, w3_e, w2_e, w1_s, w3_s, w2_s):
    global LAST_RESULT
    x = np.ascontiguousarray(np.asarray(hidden_states, np.float32).reshape(T, D))

    # ---- host routing (sharding decision) + combine coefficients ----
    gate_W = np.asarray(gate_W, np.float32)
    logits = x @ gate_W.T                       # [T, E]
    m = logits.max(axis=1, keepdims=True)
    p = np.exp(logits - m)
    probs = p / p.sum(axis=1, keepdims=True)
    order = np.argsort(-probs, axis=1, kind="stable")[:, :K]   # [T, K]

    idx = [np.where((order == e).any(axis=1))[0] for e in range(E)]
    nsz = [len(te) for te in idx]
    assert sum(nsz) == T * K
    perm = _best_perm(nsz)                      # packing order of the groups
    sizes = tuple(nsz[e] for e in perm)

    # ---- build device inputs ----------------------------------------
    xT = np.ascontiguousarray(x.T)              # [D, T] fp32
    xf_bf = xT.astype(BF16)                     # [D, T]
    xp_bf = np.empty((D, NCOL), dtype=BF16)
    off = 0
    for e in perm:
        n = len(idx[e])
        xp_bf[:, off:off + n] = xf_bf[:, idx[e]]
        off += n
    xp_bf[:, T * K:] = xf_bf                    # shared group: all tokens

    w1_e = np.asarray(w1_e, np.float32)
    w3_e = np.asarray(w3_e, np.float32)
    w2_e = np.asarray(w2_e, np.float32)
    w1_s = np.asarray(w1_s, np.float32)
    w3_s = np.asarray(w3_s, np.float32)
    # fold alpha/NS (an exact power of two) into the shared down-proj
    w2_s = np.asarray(w2_s, np.float32) * (ALPHA / NS)

    nc = _CACHE.get(sizes)
    if nc is None:
        nc = _CACHE[sizes] = _build_program(sizes)

    def _pack(w, na):
        # [NW, na*128, free] -> tile layout [NW, 128, na, free], contiguous
        nw, dd, fr = w.shape
        return np.ascontiguousarray(
            w.reshape(nw, na, 128, fr).transpose(0, 2, 1, 3)).astype(BF16)

    in_maps = []
    for c in range(N_CORES):
        fsl = slice(c * FL, (c + 1) * FL)
        w1c = np.concatenate(
            [np.ascontiguousarray(w1_e[perm][:, :, fsl]), w1_s[c:c + 1]],
            axis=0)
        w3c = np.concatenate(
            [np.ascontiguousarray(w3_e[perm][:, :, fsl]), w3_s[c:c + 1]],
            axis=0)
        w2c = np.concatenate(
            [np.ascontiguousarray(w2_e[perm][:, fsl, :]), w2_s[c:c + 1]],
            axis=0)
        in_maps.append({
            "xp": xp_bf,
            "w1": _pack(w1c, 8),
            "w3": _pack(w3c, 8),
            "w2": _pack(w2c, 4),
        })

    res = run_bass_kernel_spmd(nc, in_maps, list(range(N_CORES)))
    LAST_RESULT = res

    # ---- host combine (unshard + weighted MoE combine) --------------
    yfull = np.zeros((NIT, 128, 8, IW), np.float32)
    for c in range(N_CORES):
        yfull += res.results[c]["y"].astype(np.float32)
    # [it, p, a, t] -> [a*128+p, it*512+t] = [D, NCOL]
    yfull = np.ascontiguousarray(yfull.transpose(2, 1, 0, 3)).reshape(D, NCOL)

    outT = yfull[:, T * K:].copy()              # shared part (scales folded)
    off = 0
    for e in perm:
        te = idx[e]
        coef = ((1.0 - ALPHA) * probs[te, e]).astype(np.float32)
        outT[:, te] += yfull[:, off:off + len(te)] * coef[None, :]
        off += len(te)

    return np.ascontiguousarray(outT.T).reshape(B, S, D).astype(np.float32)
